# revision 13
# baseline (speedup 1.0000x reference)
"""LogGaborConv2d on 8 TRN2 NeuronCores.

Strategy: data-parallel over batch (8 images -> 8 cores). Key ideas:

- The log-Gabor weights depend on the 3x3 grid only through
  r^2 = x^2 + y^2, so the kernel is symmetric across its diagonal:
  w[0,1]==w[1,0], w[0,2]==w[2,0], w[1,2]==w[2,1] -> only 6 unique taps.
  With on-chip pair-sum streams S1[q] = x[q] + x[q+257] and
  S2[q] = x[q] + x[q+514], the 9 matmuls per output window collapse
  to 6 (1.5x less PE work).
- bf16 weights (host-computed from the tiny Gabor params) + bf16 input
  stream + bf16 output: halves HBM traffic on both sides (the per-core
  HBM limit of ~358 GB/s would otherwise bound the kernel) and enables
  fast weight load. Matmuls accumulate in fp32 PSUM; measured rel err
  ~4e-3 vs the fp32 reference, well under the 2e-2 gate.
- Two PE row groups: partitions 0:64 process windows 0..63 of the
  padded output stream, partitions 64:128 windows 64..128, as
  concurrent K=64 matmuls (measured steady state ~108 ns per matmul,
  i.e. the warm 2.4 GHz roofline).
- Warm-up: a burst of dummy matmuls on scratch SBUF right at kernel
  start lifts the PE HAM clock gate (1.2 -> 2.4 GHz) before the first
  input tile lands.
- Engine separation: Sync HWDGE ring carries only input prefetch;
  weight upload and output writeback go via the GPSIMD SWDGE ring.
  Pair sums are emitted in 4 chunks interleaved with the PSUM->SBUF
  copies so a long vector op never head-of-line blocks a copy (which
  would stall PSUM bank recycling and the PE).

Host side computes the 6 unique [64,128] weight blocks in numpy,
pads/shards inputs, and de-pads/gathers outputs.
"""
import math

import ml_dtypes
import numpy as np

import concourse.bacc as bacc
import concourse.bass as bass  # noqa: F401
import concourse.mybir as mybir
import concourse.tile as tile
from concourse.bass_utils import run_bass_kernel_spmd

F32 = mybir.dt.float32
BF16 = mybir.dt.bfloat16
BF16_NP = ml_dtypes.bfloat16

# problem constants
NB, C, H, W = 8, 64, 256, 256
O = 128
WP = W + 2            # padded row width
SL = (H + 2) * WP     # padded input stream length (incl. top/bottom pad rows)
OL = H * WP           # padded output stream length
NWIN = OL // 512      # 129 windows of 512
GUARD = 4             # leading guard zeros in the host-side stream
XLEN = 66576          # GUARD + SL rounded up; covers the mini tile too
TLEN = 512 * 8 + 524  # input tile: 8 windows + halo
TLEN_MINI = 512 + 524
L1 = TLEN - 257       # S1 pair-sum stream length per tile
L2 = TLEN - 514       # S2 pair-sum stream length per tile
NW_A = 64             # windows handled by partitions 0:64
N_DUMMY = 24          # warm-up matmuls

# 6 unique taps after diagonal merge. Each entry:
#   ((ky,kx), src, off): src 0=xt, 1=s1, 2=s2; off = in-tile column
#   offset for window j=0 (window j adds 512*j).
# xt-only taps first so the pair-sum streams get scheduling slack.
TAPS = [
    ((0, 0), 0, GUARD - 1),           # w00: xt at -1
    ((1, 1), 0, GUARD + WP),          # w11: xt at 258
    ((2, 2), 0, GUARD + 2 * WP + 1),  # w22: xt at 517
    ((0, 1), 1, GUARD),               # w01 (=w10): S1 at 0
    ((1, 2), 1, GUARD + WP + 1),      # w12 (=w21): S1 at 259
    ((0, 2), 2, GUARD + 1),           # w02 (=w20): S2 at 1
]


def build_kernel():
    nc = bacc.Bacc("TRN2", target_bir_lowering=False)
    x = nc.dram_tensor("x", [C, XLEN], BF16, kind="ExternalInput")
    wt_in = nc.dram_tensor("wt", [O, 6 * 128], BF16, kind="ExternalInput")
    y = nc.dram_tensor("y", [O, OL], BF16, kind="ExternalOutput")

    with tile.TileContext(nc) as tc:
        with (
            tc.tile_pool(name="wg", bufs=1) as wg,
            tc.tile_pool(name="xin", bufs=3) as xin,
            tc.tile_pool(name="s1p", bufs=3) as s1p,
            tc.tile_pool(name="s2p", bufs=3) as s2p,
            tc.tile_pool(name="outp", bufs=3) as outp,
            tc.tile_pool(name="ps", bufs=2, space="PSUM") as ps,
        ):
            # wt via the ACT engine's HWDGE ring so the Sync ring starts
            # streaming the first input tile immediately
            wt = wg.tile([O, 6 * 128], BF16)
            nc.scalar.dma_start(wt[:], wt_in[:])

            def make_tiles_dma(w0a, w0b, tlen, b_only=False, split_a=False):
                xt = xin.tile([O, TLEN], BF16, tag="xt", name="xt")
                if not b_only:
                    if split_a:
                        c0 = 512 * 2 + 524
                        nc.sync.dma_start(
                            xt[0:C, 0:c0], x[:, 512 * w0a : 512 * w0a + c0]
                        )
                        nc.sync.dma_start(
                            xt[0:C, c0:tlen],
                            x[:, 512 * w0a + c0 : 512 * w0a + tlen],
                        )
                    else:
                        nc.sync.dma_start(
                            xt[0:C, 0:tlen], x[:, 512 * w0a : 512 * w0a + tlen]
                        )
                nc.sync.dma_start(
                    xt[C:O, 0:tlen], x[:, 512 * w0b : 512 * w0b + tlen]
                )
                s1 = s1p.tile([O, L1], BF16, tag="s1", name="s1")
                s2 = s2p.tile([O, L2], BF16, tag="s2", name="s2")
                return xt, s1, s2

            def emit_sum_chunk(srcs, chunk, l1, l2, b_only=False):
                # chunk 0/1 -> s1 halves, 2/3 -> s2 halves
                xt, s1, s2 = srcs
                p0 = C if b_only else 0
                if chunk < 2:
                    st, sh, ln = s1, 257, l1
                else:
                    st, sh, ln = s2, 514, l2
                h = (ln + 1) // 2
                a, b = (0, h) if chunk % 2 == 0 else (h, ln)
                nc.vector.tensor_add(
                    st[p0:O, a:b], xt[p0:O, a:b], xt[p0:O, a + sh : b + sh]
                )

            def emit_group(wa0, na, wb0, nb, srcs, w0a, w0b, ot, oc, sub,
                           last=False):
                xt, s1, s2 = srcs
                stiles = (xt, s1, s2)
                pa = ps.tile([O, 1024], F32, tag="pa", name="pa") if na else None
                pb = ps.tile([O, 1024], F32, tag="pb", name="pb") if nb else None
                ntap = len(TAPS)
                for ti, (_, src, off) in enumerate(TAPS):
                    st = stiles[src]
                    lhs_a = wt[0:C, 128 * ti : 128 * ti + 128]
                    lhs_b = wt[C:O, 128 * ti : 128 * ti + 128]
                    first = ti == 0
                    last = ti == ntap - 1
                    for j in range(max(na, nb)):
                        if j < na:
                            o = 512 * (wa0 + j - w0a) + off
                            nc.tensor.matmul(
                                pa[:, 512 * j : 512 * j + 512],
                                lhs_a, st[0:C, o : o + 512],
                                start=first, stop=last,
                            )
                        if j < nb:
                            o = 512 * (wb0 + j - w0b) + off
                            nc.tensor.matmul(
                                pb[:, 512 * j : 512 * j + 512],
                                lhs_b, st[C:O, o : o + 512],
                                start=first, stop=last,
                            )
                # all PSUM->SBUF copies on ACT: the DVE runs only pair sums,
                # so a sum waiting on an input DMA can never head-of-line
                # block a copy (which would stall PSUM recycling and the PE)
                if na:
                    nc.scalar.copy(ot[:, oc : oc + 512 * na], pa[:, 0 : 512 * na])
                if nb:
                    if last:
                        # very last group: DVE is idle, run both copies in
                        # parallel to shorten the kernel tail
                        nc.vector.tensor_copy(
                            ot[:, 2048 + oc : 2048 + oc + 512 * nb],
                            pb[:, 0 : 512 * nb],
                        )
                    else:
                        nc.scalar.copy(
                            ot[:, 2048 + oc : 2048 + oc + 512 * nb],
                            pb[:, 0 : 512 * nb],
                        )

            # ---- prologue: tile 0 + mini tile DMAs and sums ----
            srcs0 = make_tiles_dma(0, NW_A, TLEN, split_a=True)
            srcs_next = None
            srcs_m = make_tiles_dma(0, 128, TLEN_MINI, b_only=True)
            for ch in range(4):
                emit_sum_chunk(srcs0, ch, L1, L2)
            for ch in range(4):
                emit_sum_chunk(
                    srcs_m, ch, TLEN_MINI - 257, TLEN_MINI - 514, b_only=True
                )

            srcs = srcs0
            for tblk in range(8):
                w0a = 8 * tblk
                w0b = NW_A + 8 * tblk
                if tblk < 7:
                    srcs_next = make_tiles_dma(w0a + 8, w0b + 8, TLEN)
                last_half = tblk == 7
                for half in range(2):
                    ot = outp.tile([O, 4096], BF16, tag="ot", name="ot")
                    for s in range(2):
                        sub = 2 * half + s
                        emit_group(
                            w0a + 2 * sub, 2, w0b + 2 * sub, 2, srcs,
                            w0a, w0b, ot, 1024 * s, sub,
                            last=(tblk == 7 and sub == 3),
                        )
                        if tblk < 7:
                            emit_sum_chunk(srcs_next, sub, L1, L2)
                        if last_half and half == 1:
                            # smaller final DMAs: the kernel tail waits on
                            # the last receipt, so flush per emit-group
                            nc.sync.dma_start(
                                y[:, 512 * (w0a + 2 * sub) : 512 * (w0a + 2 * sub + 2)],
                                ot[:, 1024 * s : 1024 * s + 1024],
                            )
                            nc.sync.dma_start(
                                y[:, 512 * (w0b + 2 * sub) : 512 * (w0b + 2 * sub + 2)],
                                ot[:, 2048 + 1024 * s : 2048 + 1024 * s + 1024],
                            )
                    if not (last_half and half == 1):
                        nc.sync.dma_start(
                            y[:, 512 * (w0a + 4 * half) : 512 * (w0a + 4 * half + 4)],
                            ot[:, 0:2048],
                        )
                        nc.sync.dma_start(
                            y[:, 512 * (w0b + 4 * half) : 512 * (w0b + 4 * half + 4)],
                            ot[:, 2048:4096],
                        )
                if tblk == 0:
                    # final window 128 (B row group), early so its tail hides
                    otm = outp.tile([O, 4096], BF16, tag="ot", name="otm")
                    emit_group(0, 0, 128, 1, srcs_m, 0, 128, otm, 0, 1)
                    nc.sync.dma_start(
                        y[:, 512 * 128 : 512 * 129], otm[:, 2048:2560]
                    )
                srcs = srcs_next

    nc.compile()
    return nc


_NC_CACHE = None


def _get_nc():
    global _NC_CACHE
    if _NC_CACHE is None:
        _NC_CACHE = build_kernel()
    return _NC_CACHE


def _host_weights(freq, theta, sigma, psi, f0, theta0, xg, yg):
    """6 unique [64,128] weight blocks, layout [K=128, 6*128] bf16."""
    freq = np.asarray(freq, np.float32)
    theta = np.asarray(theta, np.float32)
    sigma = np.asarray(sigma, np.float32)
    psi = np.asarray(psi, np.float32)
    f0v = float(np.asarray(f0).reshape(-1)[0])
    th0 = float(np.asarray(theta0).reshape(-1)[0])
    xg = np.asarray(xg, np.float32)
    yg = np.asarray(yg, np.float32)
    lsg = 2.0 * np.log(sigma / f0v)
    g_ang = np.exp(-((theta - th0) ** 2) / (2.0 * sigma**2))
    norm = 1.0 / (2.0 * math.pi * sigma**2)
    blocks = []
    for (ky, kx), _, _ in TAPS:
        r = math.sqrt(xg[ky, kx] ** 2 + yg[ky, kx] ** 2 + 0.001)
        g_rad = np.exp(-(((math.log(r) - math.log(f0v)) / lsg) ** 2))
        wb = g_rad * g_ang * np.cos(freq * r + psi) * norm  # [O, I]
        blocks.append(wb.T)  # [I=64, O=128]
    wt = np.concatenate(blocks, axis=1)  # [64, 768]
    return np.concatenate([wt, wt], axis=0).astype(BF16_NP)  # [128, 768]


def kernel(input_tensor, freq, theta, sigma, psi, f0, theta0, xg, yg):
    xb = np.asarray(input_tensor, np.float32).astype(BF16_NP)
    wt = _host_weights(freq, theta, sigma, psi, f0, theta0, xg, yg)
    nc = _get_nc()
    in_maps = []
    for c in range(NB):
        xp = np.zeros((C, XLEN), BF16_NP)
        view = xp[:, GUARD : GUARD + SL].reshape(C, H + 2, WP)
        view[:, 1 : H + 1, 1 : W + 1] = xb[c]
        in_maps.append({"x": xp, "wt": wt})
    res = run_bass_kernel_spmd(nc, in_maps, core_ids=list(range(NB)))
    out = np.empty((NB, O, H, W), np.float32)
    for c in range(NB):
        yv = np.asarray(res.results[c]["y"], dtype=np.float32)
        out[c] = yv.reshape(O, H, WP)[:, :, 1 : W + 1]
    return out
